# revision 7
# baseline (speedup 1.0000x reference)
"""Trainium2 Bass kernel for nn_MultiHeadAttention_44092134261443.

Reference math (B=4, S=2048, D=768, H=8, dk=96):
  q  = x @ W_q.T + b_q      -> [B,H,S,dk]
  kv = x @ W_v.T + b_v      -> k = v = kv (faithful to source bug)
  w  = q k^T / sqrt(dk); mask = pad(query-row) | causal; w[mask] = -1e9
  score = softmax(w, axis=2)   # over the QUERY axis i, per column j
  out = score @ v; out += x; layernorm(out) * gamma + beta

Sharding: 8 cores = (4 batches) x (2 head-groups of 4 heads / 384 channels).
Core c: batch c//2, channels [384*(c%2), 384*(c%2)+384).

Device-side design notes:
  * Big tensors live transposed ("T layout": channels on partitions, sequence
    on the free axis) so the softmax axis (query index i) is the free axis.
  * wT[j, i] = sum_d kvT[d, j] qT[d, i]; the pad mask rides the matmul as a
    97th contraction row (ones x -1e9*pad_row); the causal mask of the
    diagonal 128-block is added with one extra bf16 matmul (identity^T @
    tri(-1e9)); fully-causally-masked (j,i) chunks are skipped outright.
  * Softmax without max-subtraction: |w/sqrt(dk)| is tiny for this data, so
    exp never overflows, and masked entries exp() to exactly 0.  Z comes free
    from the Exp activation's accum_out.  Fully-masked columns (Z == 0; the
    reference yields uniform 1/S scores there) are fixed with a rank-1
    correction corr[d] = sum_{fm j} kv[j,d]/S added to every output column.
  * 1/Z is folded into the 96-wide kv tile (per head, per j-block) instead of
    the SxS score tile; AV accumulates outT[d, i] in PSUM over 16 j-blocks.
  * LayerNorm needs the partner core's channels only through the row moments,
    so cores exchange a [S, 2] (sum, sumsq) tensor via a pairwise AllReduce
    (16 KB) instead of gathering activations; each core then normalizes and
    writes its own 384 channels for all S rows.
"""

import math
import sys

sys.path.insert(0, "/opt/trn_rl_repo")

import numpy as np

import concourse.bass as bass
import concourse.bacc as bacc
import concourse.tile as tile
from concourse import mybir
from concourse.bass_utils import run_bass_kernel_spmd

F32 = mybir.dt.float32
F16 = mybir.dt.float16
F32R = mybir.dt.float32r
BF16 = mybir.dt.bfloat16
U8 = mybir.dt.uint8
AF = mybir.ActivationFunctionType
ALU = mybir.AluOpType

B, S, D, H = 4, 2048, 768, 8
DK = 96
HL = 4            # heads per core
CH = HL * DK      # 384 channels per core
SCALE = 1.0 / math.sqrt(DK)
NEG = -1.0e9
NSB = S // 128    # 16 sequence blocks
NKB = D // 128    # 6 contraction blocks
NIC = S // 512    # 4 i-chunks
EPS = 1e-5

TRACE = False          # test harness may flip this
TRACE_KW = {}
LAST_RESULT = None

# matmul precision mode:
#   "fp32": exact fp32 operands (4 cycles/row on the PE)
#   "fp16": fp16 operands for the projection/score/AV matmuls (1 cycle/row);
#           psum accumulation stays fp32; ~1e-4 scale-relative output error
MM_MODE = "fp32"


def _bcast_ap(handle_ap, parts, free):
    """[free] 1-D AP -> [parts, free] partition-broadcast AP (step 0)."""
    return bass.AP(tensor=handle_ap.tensor, offset=handle_ap.offset,
                   ap=[[0, parts]] + [list(p) for p in handle_ap.ap])


def build_nc():
    nc = bacc.Bacc("TRN2", target_bir_lowering=False, debug=False,
                   num_devices=8)

    MDT = F16 if MM_MODE == "fp16" else F32
    xT = nc.dram_tensor("xT", [D, S], MDT, kind="ExternalInput")
    xres = nc.dram_tensor("xres", [S, CH], F32, kind="ExternalInput")
    wqT = nc.dram_tensor("wqT", [D, CH], MDT, kind="ExternalInput")
    wvT = nc.dram_tensor("wvT", [D, CH], MDT, kind="ExternalInput")
    bq = nc.dram_tensor("bq", [CH], F32, kind="ExternalInput")
    bv = nc.dram_tensor("bv", [CH], F32, kind="ExternalInput")
    msk = nc.dram_tensor("msk", [S], U8, kind="ExternalInput")
    gam = nc.dram_tensor("gam", [CH], F32, kind="ExternalInput")
    bet = nc.dram_tensor("bet", [CH], F32, kind="ExternalInput")
    out = nc.dram_tensor("out", [S, CH], F32, kind="ExternalOutput")

    import ml_dtypes
    bf = ml_dtypes.bfloat16
    ident_c = nc.inline_tensor(np.eye(128, dtype=np.float32), name="ident_c")
    identb_c = nc.inline_tensor(np.eye(128).astype(bf), name="identb_c")
    trib_c = nc.inline_tensor(
        (np.tril(np.ones((128, 128), np.float32), -1) * NEG).astype(bf),
        name="trib_c")

    with tile.TileContext(nc) as tc:
        with (
            tc.tile_pool(name="per", bufs=1) as per,
            tc.tile_pool(name="dram", bufs=1, space="DRAM") as dram,
        ):
            # ---------- persistent tiles ----------
            kv_nat = per.tile([128, NSB, CH], F32, name="kv_nat", tag="kvn")
            ident_sb = per.tile([128, 128], F32, name="ident_sb", tag="idf")
            identb = per.tile([128, 128], BF16, name="identb", tag="idb")
            trib = per.tile([128, 128], BF16, name="trib", tag="trb")
            fmw_all = per.tile([128, NSB], F32, name="fmw_all", tag="fmw")
            s12_all = per.tile([128, NSB, 2], F32, name="s12_all", tag="s12")
            corr_sb = per.tile([96, HL], F32, name="corr_sb", tag="cor")

            nc.sync.dma_start(out=ident_sb[:], in_=ident_c[:, :])
            nc.sync.dma_start(out=identb[:], in_=identb_c[:, :])
            nc.sync.dma_start(out=trib[:], in_=trib_c[:, :])

            with (
                tc.tile_pool(name="qk", bufs=1) as qk,
            ):
                # qT/kvT per head: rows 0..95 = projections, row 96 = pad-row
                # (qT) / ones-row (kvT) to fold the pad mask into the matmul.
                qT = [qk.tile([97, S], MDT, name=f"qT{h}", tag=f"qT{h}")
                      for h in range(HL)]
                kvT = [qk.tile([97, S], MDT, name=f"kvT{h}", tag=f"kvT{h}")
                       for h in range(HL)]

                # ================= projections =================
                with (
                    tc.tile_pool(name="xw", bufs=1) as xw,
                    tc.tile_pool(name="pps", bufs=4, space="PSUM") as pps,
                ):
                    xT_sb = xw.tile([128, NKB, S], MDT, name="xT_sb",
                                    tag="xt")
                    wqT_sb = xw.tile([128, NKB, CH], MDT, name="wqT_sb",
                                     tag="wq")
                    wvT_sb = xw.tile([128, NKB, CH], MDT, name="wvT_sb",
                                     tag="wv")
                    bq_sb = xw.tile([96, HL], F32, name="bq_sb", tag="bqs")
                    bv_sb = xw.tile([96, HL], F32, name="bv_sb", tag="bvs")
                    bv_bc = xw.tile([128, CH], F32, name="bv_bc", tag="bvb")

                    xT_r = xT[:, :].rearrange("(kb p) s -> kb p s", p=128)
                    for kb in range(NKB):
                        nc.sync.dma_start(out=xT_sb[:, kb, :], in_=xT_r[kb])
                    nc.sync.dma_start(
                        out=wqT_sb[:],
                        in_=wqT[:, :].rearrange("(kb p) c -> p kb c", p=128))
                    nc.sync.dma_start(
                        out=wvT_sb[:],
                        in_=wvT[:, :].rearrange("(kb p) c -> p kb c", p=128))
                    nc.sync.dma_start(
                        out=bq_sb[:],
                        in_=bq[:].rearrange("(h p) -> p h", p=96))
                    nc.sync.dma_start(
                        out=bv_sb[:],
                        in_=bv[:].rearrange("(h p) -> p h", p=96))
                    nc.sync.dma_start(out=bv_bc[:],
                                      in_=_bcast_ap(bv[:], 128, CH))

                    # pad row: mask u8 staged into kvT0 row-96 bytes,
                    # converted+scaled into qT0 row 96, then copied around.
                    nb = 512 if MDT == F32 else 1024
                    stage_u8 = kvT[0][96:97, 0:nb].bitcast(U8)
                    nc.sync.dma_start(
                        out=stage_u8,
                        in_=msk[:].rearrange("(a s) -> a s", a=1))
                    nc.vector.tensor_copy(qT[0][96:97, :], stage_u8)
                    padneg = -60000.0 if MDT == F16 else NEG
                    nc.vector.tensor_scalar_mul(qT[0][96:97, :],
                                                qT[0][96:97, :], padneg)
                    for h in range(1, HL):
                        nc.sync.dma_start(out=qT[h][96:97, :],
                                          in_=qT[0][96:97, :])
                    for h in range(HL):
                        nc.vector.memset(kvT[h][96:97, :], 1.0)

                    # qT / kvT projections: psum [96, 512] over 6 k-blocks
                    for h in range(HL):
                        hc = slice(h * DK, (h + 1) * DK)
                        for ic in range(NIC):
                            cs = slice(ic * 512, (ic + 1) * 512)
                            pq = pps.tile([96, 512], F32, name="pq", tag="pj")
                            for kb in range(NKB):
                                nc.tensor.matmul(
                                    pq[:],
                                    wqT_sb[:, kb, hc],
                                    xT_sb[:, kb, cs],
                                    start=(kb == 0), stop=(kb == NKB - 1))
                            nc.scalar.activation(
                                out=qT[h][0:96, cs], in_=pq[:],
                                func=AF.Identity, bias=bq_sb[:, h:h + 1],
                                scale=1.0)
                            pk = pps.tile([96, 512], F32, name="pk", tag="pj")
                            for kb in range(NKB):
                                nc.tensor.matmul(
                                    pk[:],
                                    wvT_sb[:, kb, hc],
                                    xT_sb[:, kb, cs],
                                    start=(kb == 0), stop=(kb == NKB - 1))
                            nc.scalar.activation(
                                out=kvT[h][0:96, cs], in_=pk[:],
                                func=AF.Identity, bias=bv_sb[:, h:h + 1],
                                scale=1.0)

                    # kv natural layout [s, ch] for the AV side
                    for sb in range(NSB):
                        pn = pps.tile([128, CH], F32, name="pn", tag="pn")
                        for kb in range(NKB):
                            nc.tensor.matmul(
                                pn[:],
                                xT_sb[:, kb, sb * 128:(sb + 1) * 128
                                      ],
                                wvT_sb[:, kb, :],
                                start=(kb == 0), stop=(kb == NKB - 1))
                        nc.vector.tensor_tensor(out=kv_nat[:, sb, :],
                                                in0=pn[:], in1=bv_bc[:],
                                                op=ALU.add)

                # ================= attention =================
                with (
                    tc.tile_pool(name="ynp", bufs=1) as ynp,
                    tc.tile_pool(name="yt", bufs=1) as yt,
                ):
                    y_nat = ynp.tile([128, NSB, CH], F32, name="y_nat",
                                     tag="ynt")
                    yT = [yt.tile([96, S], F32, name=f"yT{h}", tag=f"yT{h}")
                          for h in range(HL)]

                    with (
                        tc.tile_pool(name="att", bufs=1) as att,
                        tc.tile_pool(name="wps", bufs=2,
                                     space="PSUM") as wps,
                        tc.tile_pool(name="ops", bufs=1,
                                     space="PSUM") as ops,
                    ):
                        for h in range(HL):
                            hc = slice(h * DK, (h + 1) * DK)
                            outp = ops.tile([96, S], F32, name="outp",
                                            tag="avp")
                            for jb in range(NSB):
                                ic0 = jb // 4
                                j0 = jb * 128
                                eT = att.tile([128, S], MDT, name="eT",
                                              tag="eT", bufs=2)
                                zs = []
                                for half in range(2):
                                    lo, hi = half * 1024, (half + 1) * 1024
                                    if j0 >= hi:
                                        continue
                                    w_ps = wps.tile([128, 1024], F32,
                                                    name="w_ps", tag="wt")
                                    diag = (j0 >= lo)
                                    for g in range(max(ic0, 2 * half),
                                                   2 * (half + 1)):
                                        c0 = g * 512
                                        nc.tensor.matmul(
                                            w_ps[:, c0 - lo:c0 - lo + 512],
                                            kvT[h][:, j0:j0 + 128
                                                   ],
                                            qT[h][:, c0:c0 + 512
                                                  ],
                                            start=True,
                                            stop=not (diag and g == ic0))
                                        if diag and g == ic0:
                                            nc.tensor.matmul(
                                                w_ps[:, j0 - lo:j0 - lo + 128],
                                                identb[:], trib[:],
                                                start=False, stop=True)
                                    a0 = max(j0, lo)
                                    z = att.tile([128, 1], F32, name="z",
                                                 tag="z", bufs=8)
                                    nc.scalar.activation(
                                        out=eT[:, a0:hi],
                                        in_=w_ps[:, a0 - lo:hi - lo],
                                        func=AF.Exp, bias=0.0, scale=SCALE,
                                        accum_out=z[:])
                                    zs.append(z)

                                if len(zs) == 2:
                                    zt = att.tile([128, 1], F32, name="zt",
                                                  tag="z", bufs=8)
                                    nc.vector.tensor_scalar_add(
                                        zt[:], zs[0][:], zs[1][:])
                                else:
                                    zt = zs[0]
                                isfm = att.tile([128, 1], F32, name="isfm",
                                                tag="z", bufs=8)
                                nc.vector.tensor_scalar(
                                    out=isfm[:], in0=zt[:], scalar1=0.0,
                                    scalar2=None, op0=ALU.is_equal)
                                z2 = att.tile([128, 1], F32, name="z2",
                                              tag="z", bufs=8)
                                nc.vector.tensor_scalar_add(z2[:], zt[:],
                                                            isfm[:])
                                rz = att.tile([128, 1], F32, name="rz",
                                              tag="z", bufs=8)
                                nc.vector.reciprocal(out=rz[:], in_=z2[:])
                                if h == 0:
                                    nc.vector.tensor_scalar_mul(
                                        fmw_all[:, jb:jb + 1], isfm[:],
                                        1.0 / S)

                                kvs = att.tile([128, DK], MDT, name="kvs",
                                               tag="kvs", bufs=3)
                                nc.vector.tensor_scalar_mul(
                                    kvs[:], kv_nat[:, jb, hc], rz[:])

                                if jb % 4 != 0:
                                    nc.gpsimd.memset(eT[:, ic0 * 512:j0], 0.0)

                                for g in range(ic0, NIC):
                                    nc.tensor.matmul(
                                        outp[:, g * 512:(g + 1) * 512],
                                        kvs[:],
                                        eT[:, g * 512:(g + 1) * 512
                                           ],
                                        start=(jb == 0),
                                        stop=(jb == min(NSB - 1, 4 * g + 3)))

                            if h == 0:
                                # rank-1 fully-masked-column correction
                                for hh in range(HL):
                                    cp = wps.tile([96, 1], F32, name="cp",
                                                  tag="wt")
                                    for jb in range(NSB):
                                        nc.tensor.matmul(
                                            cp[:],
                                            kv_nat[:, jb,
                                                   hh * DK:(hh + 1) * DK
                                                   ],
                                            fmw_all[:, jb:jb + 1
                                                    ],
                                            start=(jb == 0),
                                            stop=(jb == NSB - 1))
                                    nc.vector.tensor_copy(
                                        corr_sb[:, hh:hh + 1], cp[:])

                            nc.vector.tensor_scalar_add(
                                yT[h][:, :], outp[:, :], corr_sb[:, h:h + 1])

                    # ============ transpose + residual + moments ============
                    with (
                        tc.tile_pool(name="fin", bufs=1) as fin,
                        tc.tile_pool(name="tps", bufs=4, space="PSUM") as tps,
                    ):
                        for sb in range(NSB):
                            for h in range(HL):
                                pt = tps.tile([128, 96], F32, name="pt",
                                              tag="pt")
                                nc.tensor.transpose(
                                    pt[:], yT[h][:, sb * 128:(sb + 1) * 128],
                                    ident_sb[0:96, 0:96])
                                nc.vector.tensor_copy(
                                    y_nat[:, sb, h * DK:(h + 1) * DK], pt[:])

                        xres_r = xres[:, :].rearrange("(sb p) c -> sb p c",
                                                      p=128)
                        for sb in range(NSB):
                            xn = fin.tile([128, CH], F32, name="xn", tag="xn",
                                          bufs=2)
                            nc.sync.dma_start(out=xn[:], in_=xres_r[sb])
                            nc.vector.scalar_tensor_tensor(
                                out=y_nat[:, sb, :], in0=y_nat[:, sb, :],
                                scalar=0.0, in1=xn[:], op0=ALU.bypass,
                                op1=ALU.add, accum_out=s12_all[:, sb, 0:1])
                            sq = fin.tile([128, CH], F32, name="sq", tag="sq",
                                          bufs=2)
                            nc.scalar.activation(
                                out=sq[:], in_=y_nat[:, sb, :],
                                func=AF.Square, bias=0.0, scale=1.0,
                                accum_out=s12_all[:, sb, 1:2])

                        # ---------- pairwise moment AllReduce ----------
                        s12_d = dram.tile([S, 2], F32, name="s12_d",
                                          tag="s12d")
                        s12_r = dram.tile([S, 2], F32, name="s12_r",
                                          tag="s12r")
                        nc.sync.dma_start(
                            out=s12_d.rearrange("(sb p) t -> p sb t", p=128),
                            in_=s12_all[:])
                        nc.gpsimd.collective_compute(
                            "AllReduce", ALU.add,
                            replica_groups=[[0, 1], [2, 3], [4, 5], [6, 7]],
                            ins=[s12_d.opt()], outs=[s12_r.opt()])
                        s12s = fin.tile([128, NSB, 2], F32, name="s12s",
                                        tag="s1s")
                        nc.sync.dma_start(
                            out=s12s[:],
                            in_=s12_r.rearrange("(sb p) t -> p sb t", p=128))

                        # ---------- layernorm ----------
                        gam_bc = fin.tile([128, CH], F32, name="gam_bc",
                                          tag="gb")
                        bet_bc = fin.tile([128, CH], F32, name="bet_bc",
                                          tag="bb")
                        nc.sync.dma_start(out=gam_bc[:],
                                          in_=_bcast_ap(gam[:], 128, CH))
                        nc.sync.dma_start(out=bet_bc[:],
                                          in_=_bcast_ap(bet[:], 128, CH))

                        negmu = fin.tile([128, NSB], F32, name="negmu",
                                         tag="nmu")
                        msq = fin.tile([128, NSB], F32, name="msq", tag="msq")
                        mu2 = fin.tile([128, NSB], F32, name="mu2", tag="mu2")
                        var = fin.tile([128, NSB], F32, name="var", tag="var")
                        rstd = fin.tile([128, NSB], F32, name="rstd",
                                        tag="rst")
                        nc.vector.tensor_scalar_mul(negmu[:], s12s[:, :, 0],
                                                    -1.0 / D)
                        nc.vector.tensor_scalar_mul(msq[:], s12s[:, :, 1],
                                                    1.0 / D)
                        nc.vector.tensor_tensor(out=mu2[:], in0=negmu[:],
                                                in1=negmu[:], op=ALU.mult)
                        nc.vector.tensor_tensor(out=var[:], in0=msq[:],
                                                in1=mu2[:], op=ALU.subtract)
                        eps_col = fin.tile([128, 1], F32, name="eps_col",
                                           tag="eps")
                        nc.vector.memset(eps_col[:], EPS)
                        nc.scalar.activation(out=var[:], in_=var[:],
                                             func=AF.Sqrt, bias=eps_col[:],
                                             scale=1.0)
                        nc.vector.reciprocal(out=rstd[:], in_=var[:])

                        out_r = out[:, :].rearrange("(sb p) c -> sb p c",
                                                    p=128)
                        for sb in range(NSB):
                            t1 = fin.tile([128, CH], F32, name="t1", tag="t1",
                                          bufs=2)
                            nc.vector.tensor_scalar(
                                out=t1[:], in0=y_nat[:, sb, :],
                                scalar1=negmu[:, sb:sb + 1],
                                scalar2=rstd[:, sb:sb + 1],
                                op0=ALU.add, op1=ALU.mult)
                            t2 = fin.tile([128, CH], F32, name="t2", tag="t2",
                                          bufs=2)
                            nc.gpsimd.tensor_tensor(out=t2[:], in0=t1[:],
                                                    in1=gam_bc[:],
                                                    op=ALU.mult)
                            t3 = fin.tile([128, CH], F32, name="t3", tag="t3",
                                          bufs=2)
                            nc.vector.tensor_tensor(out=t3[:], in0=t2[:],
                                                    in1=bet_bc[:], op=ALU.add)
                            nc.sync.dma_start(out=out_r[sb], in_=t3[:])
    nc.finalize()
    return nc


_NC_CACHE = []


def _get_nc():
    if not _NC_CACHE:
        _NC_CACHE.append(build_nc())
    return _NC_CACHE[0]


def shard_inputs(x, attention_mask, W_q, b_q, W_v, b_v, gamma, beta):
    x = np.asarray(x, np.float32)
    attention_mask = np.asarray(attention_mask)
    W_q = np.asarray(W_q, np.float32)
    b_q = np.asarray(b_q, np.float32)
    W_v = np.asarray(W_v, np.float32)
    b_v = np.asarray(b_v, np.float32)
    gamma = np.asarray(gamma, np.float32)
    beta = np.asarray(beta, np.float32)
    mdt = np.float16 if MM_MODE == "fp16" else np.float32
    WqT = np.ascontiguousarray(W_q.T.astype(mdt))
    WvT = np.ascontiguousarray(W_v.T.astype(mdt))
    in_maps = []
    for c in range(8):
        b = c // 2
        ch0 = (c % 2) * CH
        in_maps.append({
            "xT": np.ascontiguousarray(x[b].T.astype(mdt)),
            "xres": np.ascontiguousarray(x[b][:, ch0:ch0 + CH]),
            "wqT": np.ascontiguousarray(WqT[:, ch0:ch0 + CH]),
            "wvT": np.ascontiguousarray(WvT[:, ch0:ch0 + CH]),
            "bq": np.ascontiguousarray(b_q[ch0:ch0 + CH]),
            "bv": np.ascontiguousarray(b_v[ch0:ch0 + CH]),
            "msk": np.ascontiguousarray(
                attention_mask[b, :, 0].astype(np.uint8)),
            "gam": np.ascontiguousarray(gamma[ch0:ch0 + CH]),
            "bet": np.ascontiguousarray(beta[ch0:ch0 + CH]),
        })
    return in_maps


def assemble_output(results):
    full = np.empty((B, S, D), np.float32)
    for c in range(8):
        b = c // 2
        ch0 = (c % 2) * CH
        full[b, :, ch0:ch0 + CH] = results[c]["out"]
    return full


def kernel(**inputs):
    global LAST_RESULT
    in_maps = shard_inputs(**inputs)
    nc = _get_nc()
    res = run_bass_kernel_spmd(nc, in_maps, core_ids=list(range(8)),
                               trace=TRACE, **TRACE_KW)
    LAST_RESULT = res
    return assemble_output(res.results)


if __name__ == "__main__":
    nc = _get_nc()
    print("built OK:",
          sum(len(bb.instructions) for bb in nc.main_func.blocks),
          "instructions")
